# revision 2
# baseline (speedup 1.0000x reference)
"""LIF fully-connected neuron layer on 8 Trainium2 NeuronCores.

reference semantics (per sample b, hidden unit h):
    x[b,t,h] = sum_d input[b,t,d] * W[h,d] + bias[h]
    m_t   = mem_{t-1} + x_t
    spike = m_t > THRESH
    mem_t = m_t * (1-spike) * DECAY
    out[b,t,h] = spike

Strategy:
  - Data-parallel over batch: core c handles samples [8c, 8c+8).
  - Host pre-transposes input to [d, t, b] so matmul operands load naturally
    (contraction dim d on partitions) -- zero on-device transposes.
  - Matmul in float32r (full-rate fp32 PE mode, 1 cycle/row at >=256 moving
    cols vs 4 for plain fp32).  W is loaded as 8 per-k-tile tiles and the
    input window DMA is split per k-tile so the PE starts ~2us in.
  - PSUM: one bank per h-tile (8 banks), window w accumulates in half w%2 of
    its bank; groups are sequential per bank, parallel across banks.
  - ScalarE copies PSUM->SBUF with per-partition bias add (Identity act).
  - Scan: one fused custom DVE op per timestep over [128, 64] lanes
    (lane = (h_tile, b), partition = h_lo):
        u' = m * (m <= TH),  m = u*DECAY + x_t
    u' is the pre-decay post-reset membrane; spikes are derived in 64-step
    batches as (u' == 0) on GpSimd into uint8 and DMA'd out.  (u'==0 with no
    spike requires the membrane to be exactly 0.0 -- measure-zero.)
  - Host reassembles [B, T, H] float32 from the device uint8 layout.
"""

import numpy as np

# ---- problem constants (hardcoded per contest contract) ----
B, T, D, H = 64, 512, 1024, 1024
N_CORES = 8
B_L = B // N_CORES            # 8 samples per core
P = 128                       # partitions
DT, HT = D // P, H // P       # 8 k-tiles, 8 h-tiles
WT = 32                       # timesteps per matmul window
NW = T // WT                  # 16 windows
NCOL = WT * B_L               # 256 moving columns per window
F = HT * B_L                  # 64 scan lanes in free dim
BLK = 64                      # timesteps per spike/output block
NB = T // BLK                 # 8 output blocks

DECAY = 200.0 / 255.0
THRESH = 0.3

_CACHE = {}


def _register_lif_op():
    from concourse.dve_spec import Spec, Src0, Src1, C0, C1, lower
    from concourse.dve_ops import (
        DveOp, OPS, CUSTOM_DVE_SPECS, _SUB_OPCODE_FOR_NAME, _CUSTOM_DVE_ROW_BASE,
    )
    from concourse.dve_uop import DveOpSpec

    name = "LIF_STEP_ANT"
    for op in OPS:
        if op.name == name:
            return op

    m = Src0 * C0 + Src1
    body = (m <= C1) * m

    def ref(in0, in1, s0, s1, imm2):
        mm = (in0 * np.float32(s0) + in1).astype(np.float32)
        return (mm * (mm <= np.float32(s1))).astype(np.float32)

    spec = Spec(body=body, reference=ref)
    opcode = _CUSTOM_DVE_ROW_BASE + len(OPS)
    shas = {}
    for ver in ("v3", "v4"):
        uops = lower(spec, ver=ver)
        shas[ver] = DveOpSpec(name=name, opcode=opcode, uops=uops, rd1_en=True).sha(ver)
    op = DveOp(name, spec, subdim=False, uops_sha=shas)
    OPS.append(op)
    _SUB_OPCODE_FOR_NAME[name] = opcode
    CUSTOM_DVE_SPECS[name] = spec
    return op


def _build():
    if "nc" in _CACHE:
        return _CACHE["nc"]
    from contextlib import ExitStack
    import concourse.bacc as bacc
    import concourse.tile as tile
    from concourse import mybir

    lif_op = _register_lif_op()

    nc = bacc.Bacc("TRN2", target_bir_lowering=False, debug=False,
                   num_devices=N_CORES)
    f32 = mybir.dt.float32
    f32r = mybir.dt.float32r
    u8 = mybir.dt.uint8
    xin_d = nc.dram_tensor("xin", [D, T * B_L], f32r, kind="ExternalInput").ap()
    wt_d = nc.dram_tensor("wt", [D, H], f32r, kind="ExternalInput").ap()
    bias_d = nc.dram_tensor("bias", [P, HT], f32, kind="ExternalInput").ap()
    out_d = nc.dram_tensor("out", [NB, P, BLK * F], u8, kind="ExternalOutput").ap()

    with tile.TileContext(nc) as tc, ExitStack() as ctx:
        const_pool = ctx.enter_context(tc.tile_pool(name="const", bufs=1))
        rhs_pool = ctx.enter_context(tc.tile_pool(name="rhs", bufs=3))
        xs_pool = ctx.enter_context(tc.tile_pool(name="xs", bufs=2))
        psum_pool = ctx.enter_context(tc.tile_pool(name="psum", bufs=1, space="PSUM"))
        spk_pool = ctx.enter_context(tc.tile_pool(name="spk", bufs=2))

        xin_r = xin_d.rearrange("(dt p) n -> p dt n", dt=DT)
        wt_r = wt_d.rearrange("(dt p) h -> dt p h", dt=DT)

        # --- constants: W as 8 per-k-tile tiles so matmuls start early ---
        bias_s = const_pool.tile([P, HT], f32)
        nc.sync.dma_start(bias_s[:], bias_d)
        wt_s = []
        for dt in range(DT):
            w_t = const_pool.tile([P, H], f32r, name=f"wt{dt}")
            nc.sync.dma_start(w_t[:], wt_r[dt])
            wt_s.append(w_t)

        # --- membrane ring: 128 slots of F lanes; slot t%128 = u after step t
        ring = const_pool.tile([P, 2 * BLK * F], f32)
        nc.vector.memset(ring[:, (2 * BLK - 1) * F:], 0.0)

        # --- PSUM: one bank per h-tile; window w lives in half w%2 ---
        pt = [psum_pool.tile([P, 2 * NCOL], f32, name=f"pt{ht}") for ht in range(HT)]

        for w in range(NW):
            half = (w % 2) * NCOL
            # load input^T window per k-tile: [d_lo, (dt, 32t x 8b)]
            rhs = rhs_pool.tile([P, DT * NCOL], f32r)
            for dt in range(DT):
                nc.sync.dma_start(
                    rhs[:, dt * NCOL:(dt + 1) * NCOL],
                    xin_r[:, dt, w * NCOL:(w + 1) * NCOL],
                )
            # matmul: k-step outer so dt=0 starts once its W/input slices land;
            # 8 h-tile groups accumulate in parallel across the 8 banks.
            for dt in range(DT):
                for ht in range(HT):
                    nc.tensor.matmul(
                        pt[ht][:, half:half + NCOL],
                        wt_s[dt][:, ht * P: ht * P + P],
                        rhs[:, dt * NCOL:(dt + 1) * NCOL],
                        start=(dt == 0),
                        stop=(dt == DT - 1),
                    )
            # PSUM -> SBUF with bias add (ScalarE)
            xs = xs_pool.tile([P, HT * NCOL], f32)        # [p, (ht, t32, b8)]
            for ht in range(HT):
                nc.scalar.activation(
                    xs[:, ht * NCOL:(ht + 1) * NCOL],
                    pt[ht][:, half:half + NCOL],
                    mybir.ActivationFunctionType.Identity,
                    bias=bias_s[:, ht:ht + 1],
                    scale=1.0,
                )
            # scan: one fused DVE op per timestep
            xs_r = xs[:].rearrange("p (ht t b) -> p t ht b", ht=HT, t=WT, b=B_L)
            for tt in range(WT):
                t = w * WT + tt
                s_out = (t % (2 * BLK)) * F
                s_in = ((t - 1) % (2 * BLK)) * F
                nc.vector._custom_dve(
                    lif_op,
                    out=ring[:, s_out:s_out + F],
                    in0=ring[:, s_in:s_in + F],
                    in1=xs_r[:, tt],
                    s0=DECAY,
                    s1=THRESH,
                )
            # every 2 windows: derive spikes for the finished 64-step block
            # on GpSimd (uint8), keeping the DVE free for the scan chain.
            if w % 2 == 1:
                blk = w // 2
                rhalf = (blk % 2) * BLK * F
                spk = spk_pool.tile([P, BLK * F], u8)
                nc.gpsimd.tensor_scalar(
                    out=spk[:], in0=ring[:, rhalf:rhalf + BLK * F],
                    scalar1=0.0, scalar2=None, op0=mybir.AluOpType.is_equal,
                )
                nc.sync.dma_start(out_d[blk], spk[:])

    nc.compile()
    _CACHE["nc"] = nc
    return nc


def _prep_inputs(input_data, W, b):
    """Full [B,T,D] inputs -> per-core in_maps (host-side shard + transpose)."""
    input_data = np.asarray(input_data, dtype=np.float32)
    W = np.asarray(W, dtype=np.float32)
    b = np.asarray(b, dtype=np.float32)
    wt = np.ascontiguousarray(W.T)                       # [d, h]
    bias = np.ascontiguousarray(b.reshape(HT, P).T)      # [h_lo, ht]
    in_maps = []
    for c in range(N_CORES):
        xc = input_data[c * B_L:(c + 1) * B_L]           # [8, T, D]
        xin = np.ascontiguousarray(xc.transpose(2, 1, 0)).reshape(D, T * B_L)
        in_maps.append({"xin": xin, "wt": wt, "bias": bias})
    return in_maps


def _decode_outputs(results):
    """Per-core uint8 'out' buffers -> full [B,T,H] float32."""
    outs = []
    for c in range(N_CORES):
        o = results[c]["out"]                            # [NB, P, BLK*F] u8
        o = o.reshape(NB, P, BLK, HT, B_L)               # [blk, h_lo, t, ht, b]
        o = o.transpose(4, 0, 2, 3, 1).reshape(B_L, T, H)
        outs.append(o.astype(np.float32))
    return np.ascontiguousarray(np.concatenate(outs, axis=0))


def kernel(input_data, W, b):
    from concourse.bass_utils import run_bass_kernel_spmd

    nc = _build()
    in_maps = _prep_inputs(input_data, W, b)
    res = run_bass_kernel_spmd(nc, in_maps, core_ids=list(range(N_CORES)))
    return _decode_outputs(res.results)


# revision 3
# speedup vs baseline: 3.4565x; 3.4565x over previous
"""LIF fully-connected neuron layer on 8 Trainium2 NeuronCores.

reference semantics (per sample b, hidden unit h):
    x[b,t,h] = sum_d input[b,t,d] * W[h,d] + bias[h]
    m_t   = mem_{t-1} + x_t
    spike = m_t > THRESH
    mem_t = m_t * (1-spike) * DECAY
    out[b,t,h] = spike

Strategy:
  - Data-parallel over batch: core c handles samples [8c, 8c+8).
  - Host pre-transposes input to [d, t, b] so matmul operands load naturally
    (contraction dim d on partitions) -- zero on-device transposes.
  - Matmul in float32r (full-rate fp32 PE mode, 1 cycle/row at >=256 moving
    cols vs 4 for plain fp32).  W is loaded as 8 per-k-tile tiles and the
    input window DMA is split per k-tile so the PE starts ~2us in.
  - PSUM: one bank per h-tile (8 banks), window w accumulates in half w%2 of
    its bank; groups are sequential per bank, parallel across banks.
  - ScalarE copies PSUM->SBUF with per-partition bias add (Identity act).
  - Scan: one fused custom DVE op per timestep over [128, 64] lanes
    (lane = (h_tile, b), partition = h_lo):
        u' = m * (m <= TH),  m = u*DECAY + x_t
    u' is the pre-decay post-reset membrane; spikes are derived in 64-step
    batches as (u' == 0) on GpSimd into uint8 and DMA'd out.  (u'==0 with no
    spike requires the membrane to be exactly 0.0 -- measure-zero.)
  - Host reassembles [B, T, H] float32 from the device uint8 layout.
"""

import numpy as np

# ---- problem constants (hardcoded per contest contract) ----
B, T, D, H = 64, 512, 1024, 1024
N_CORES = 8
B_L = B // N_CORES            # 8 samples per core
P = 128                       # partitions
DT, HT = D // P, H // P       # 8 k-tiles, 8 h-tiles
WT = 32                       # timesteps per matmul window
NW = T // WT                  # 16 windows
NCOL = WT * B_L               # 256 moving columns per window
F = HT * B_L                  # 64 scan lanes in free dim
BLK = 64                      # timesteps per spike/output block
NB = T // BLK                 # 8 output blocks

DECAY = 200.0 / 255.0
THRESH = 0.3

_CACHE = {}


def _register_lif_op():
    from concourse.dve_spec import Spec, Src0, Src1, C0, C1, lower
    from concourse.dve_ops import (
        DveOp, OPS, CUSTOM_DVE_SPECS, _SUB_OPCODE_FOR_NAME, _CUSTOM_DVE_ROW_BASE,
    )
    from concourse.dve_uop import DveOpSpec

    name = "LIF_STEP_ANT"
    for op in OPS:
        if op.name == name:
            return op

    m = Src0 * C0 + Src1
    body = (m <= C1) * m

    def ref(in0, in1, s0, s1, imm2):
        mm = (in0 * np.float32(s0) + in1).astype(np.float32)
        return (mm * (mm <= np.float32(s1))).astype(np.float32)

    spec = Spec(body=body, reference=ref)
    opcode = _CUSTOM_DVE_ROW_BASE + len(OPS)
    shas = {}
    for ver in ("v3", "v4"):
        uops = lower(spec, ver=ver)
        shas[ver] = DveOpSpec(name=name, opcode=opcode, uops=uops, rd1_en=True).sha(ver)
    op = DveOp(name, spec, subdim=False, uops_sha=shas)
    OPS.append(op)
    _SUB_OPCODE_FOR_NAME[name] = opcode
    CUSTOM_DVE_SPECS[name] = spec
    return op


def _build():
    if "nc" in _CACHE:
        return _CACHE["nc"]
    from contextlib import ExitStack
    import concourse.bacc as bacc
    import concourse.tile as tile
    from concourse import mybir

    lif_op = _register_lif_op()

    nc = bacc.Bacc("TRN2", target_bir_lowering=False, debug=False,
                   num_devices=N_CORES)
    f32 = mybir.dt.float32
    f32r = mybir.dt.float32r
    u8 = mybir.dt.uint8
    xin_d = nc.dram_tensor("xin", [D, T * B_L], f32r, kind="ExternalInput").ap()
    wt_d = nc.dram_tensor("wt", [D, H], f32r, kind="ExternalInput").ap()
    bias_d = nc.dram_tensor("bias", [P, HT], f32, kind="ExternalInput").ap()
    out_d = nc.dram_tensor("out", [NB, P, BLK * F], u8, kind="ExternalOutput").ap()

    with tile.TileContext(nc) as tc, ExitStack() as ctx:
        const_pool = ctx.enter_context(tc.tile_pool(name="const", bufs=1))
        rhs_pool = ctx.enter_context(tc.tile_pool(name="rhs", bufs=3))
        xs_pool = ctx.enter_context(tc.tile_pool(name="xs", bufs=2))
        psum_pool = ctx.enter_context(tc.tile_pool(name="psum", bufs=1, space="PSUM"))
        spk_pool = ctx.enter_context(tc.tile_pool(name="spk", bufs=2))

        xin_r = xin_d.rearrange("(dt p) n -> p dt n", dt=DT)
        wt_r = wt_d.rearrange("(dt p) h -> dt p h", dt=DT)

        # --- constants: W as 8 per-k-tile tiles so matmuls start early ---
        bias_s = const_pool.tile([P, HT], f32)
        nc.sync.dma_start(bias_s[:], bias_d)
        wt_s = []
        for dt in range(DT):
            w_t = const_pool.tile([P, H], f32r, name=f"wt{dt}")
            nc.sync.dma_start(w_t[:], wt_r[dt])
            wt_s.append(w_t)

        # --- membrane ring: 128 slots of F lanes; slot t%128 = u after step t
        ring = const_pool.tile([P, 2 * BLK * F], f32)
        nc.vector.memset(ring[:, (2 * BLK - 1) * F:], 0.0)

        # --- PSUM: one bank per h-tile; window w lives in half w%2 ---
        pt = [psum_pool.tile([P, 2 * NCOL], f32, name=f"pt{ht}") for ht in range(HT)]

        for w in range(NW):
            half = (w % 2) * NCOL
            # load input^T window per k-tile: [d_lo, (dt, 32t x 8b)]
            rhs = rhs_pool.tile([P, DT * NCOL], f32r)
            for dt in range(DT):
                nc.sync.dma_start(
                    rhs[:, dt * NCOL:(dt + 1) * NCOL],
                    xin_r[:, dt, w * NCOL:(w + 1) * NCOL],
                )
            # matmul: k-step outer so dt=0 starts once its W/input slices land;
            # 8 h-tile groups accumulate in parallel across the 8 banks.
            for dt in range(DT):
                for ht in range(HT):
                    nc.tensor.matmul(
                        pt[ht][:, half:half + NCOL],
                        wt_s[dt][:, ht * P: ht * P + P],
                        rhs[:, dt * NCOL:(dt + 1) * NCOL],
                        start=(dt == 0),
                        stop=(dt == DT - 1),
                    )
            # PSUM -> SBUF with bias add (ScalarE)
            xs = xs_pool.tile([P, HT * NCOL], f32)        # [p, (ht, t32, b8)]
            for ht in range(HT):
                nc.scalar.activation(
                    xs[:, ht * NCOL:(ht + 1) * NCOL],
                    pt[ht][:, half:half + NCOL],
                    mybir.ActivationFunctionType.Identity,
                    bias=bias_s[:, ht:ht + 1],
                    scale=1.0,
                )
            # scan: one fused DVE op per timestep
            xs_r = xs[:].rearrange("p (ht t b) -> p t ht b", ht=HT, t=WT, b=B_L)
            for tt in range(WT):
                t = w * WT + tt
                s_out = (t % (2 * BLK)) * F
                s_in = ((t - 1) % (2 * BLK)) * F
                nc.vector._custom_dve(
                    lif_op,
                    out=ring[:, s_out:s_out + F],
                    in0=ring[:, s_in:s_in + F],
                    in1=xs_r[:, tt],
                    s0=DECAY,
                    s1=THRESH,
                )
            # every 2 windows: derive spikes for the finished 64-step block
            # (uint8 halves the output DMA traffic vs f32).
            if w % 2 == 1:
                blk = w // 2
                rhalf = (blk % 2) * BLK * F
                spk = spk_pool.tile([P, BLK * F], u8)
                nc.vector.tensor_scalar(
                    out=spk[:], in0=ring[:, rhalf:rhalf + BLK * F],
                    scalar1=0.0, scalar2=None, op0=mybir.AluOpType.is_equal,
                )
                nc.sync.dma_start(out_d[blk], spk[:])

    nc.compile()
    _CACHE["nc"] = nc
    return nc


def _prep_inputs(input_data, W, b):
    """Full [B,T,D] inputs -> per-core in_maps (host-side shard + transpose)."""
    input_data = np.asarray(input_data, dtype=np.float32)
    W = np.asarray(W, dtype=np.float32)
    b = np.asarray(b, dtype=np.float32)
    wt = np.ascontiguousarray(W.T)                       # [d, h]
    bias = np.ascontiguousarray(b.reshape(HT, P).T)      # [h_lo, ht]
    in_maps = []
    for c in range(N_CORES):
        xc = input_data[c * B_L:(c + 1) * B_L]           # [8, T, D]
        xin = np.ascontiguousarray(xc.transpose(2, 1, 0)).reshape(D, T * B_L)
        in_maps.append({"xin": xin, "wt": wt, "bias": bias})
    return in_maps


def _decode_outputs(results):
    """Per-core uint8 'out' buffers -> full [B,T,H] float32."""
    outs = []
    for c in range(N_CORES):
        o = results[c]["out"]                            # [NB, P, BLK*F] u8
        o = o.reshape(NB, P, BLK, HT, B_L)               # [blk, h_lo, t, ht, b]
        o = o.transpose(4, 0, 2, 3, 1).reshape(B_L, T, H)
        outs.append(o.astype(np.float32))
    return np.ascontiguousarray(np.concatenate(outs, axis=0))


def kernel(input_data, W, b):
    from concourse.bass_utils import run_bass_kernel_spmd

    nc = _build()
    in_maps = _prep_inputs(input_data, W, b)
    res = run_bass_kernel_spmd(nc, in_maps, core_ids=list(range(N_CORES)))
    return _decode_outputs(res.results)


# revision 6
# speedup vs baseline: 3.6331x; 1.0511x over previous
"""LIF fully-connected neuron layer on 8 Trainium2 NeuronCores.

reference semantics (per sample b, hidden unit h):
    x[b,t,h] = sum_d input[b,t,d] * W[h,d] + bias[h]
    m_t   = mem_{t-1} + x_t
    spike = m_t > THRESH
    mem_t = m_t * (1-spike) * DECAY
    out[b,t,h] = spike

Strategy:
  - Data-parallel over batch: core c handles samples [8c, 8c+8).
  - Host pre-transposes input to [d, t, b] so matmul operands load naturally
    (contraction dim d on partitions) -- zero on-device transposes.
  - Matmul in float32r (full-rate fp32 PE mode, 1 cycle/row at >=256 moving
    cols vs 4 for plain fp32), 512-col windows (64 timesteps x 8 samples).
  - PSUM: one full bank per h-tile.  Window 0 runs k-outer so the first
    matmul starts as soon as the first W k-tile lands; later windows run
    h-outer so each bank's copy-out completes long before the next window's
    group reopens it.
  - ScalarE copies PSUM->SBUF with per-partition bias add (Identity act).
  - Scan: one fused custom DVE op per timestep over [128, 64] lanes
    (lane = (h_tile, b), partition = h_lo), ring stores the PRE-reset
    membrane m_t:
        m_t = (m_{t-1} * (m_{t-1} <= TH)) * DECAY + x_t
  - Spikes: ScalarE saturated sigmoid  sigmoid(S*(m - TH)) -> {0.0, 1.0}
    written as uint8 in 32-step chunks (short pipeline tail, 4x less
    output DMA than f32), DVE runs nothing but the scan chain.
  - Host reassembles [B, T, H] float32 from the device uint8 layout.
"""

import numpy as np

# ---- problem constants (hardcoded per contest contract) ----
B, T, D, H = 64, 512, 1024, 1024
N_CORES = 8
B_L = B // N_CORES            # 8 samples per core
P = 128                       # partitions
DT, HT = D // P, H // P       # 8 k-tiles, 8 h-tiles
WT = 64                       # timesteps per matmul window
NW = T // WT                  # 8 windows
NCOL = WT * B_L               # 512 moving columns per window
F = HT * B_L                  # 64 scan lanes in free dim
BLK = 32                      # timesteps per spike/output chunk
NB = T // BLK                 # 16 output chunks
RING = 128                    # membrane ring slots (2 windows)

DECAY = 200.0 / 255.0
THRESH = 0.3
SIG_SCALE = 1e30              # heaviside via saturated sigmoid

_CACHE = {}


def _register_lif_op():
    from concourse.dve_spec import Spec, Src0, Src1, C0, C1, lower
    from concourse.dve_ops import (
        DveOp, OPS, CUSTOM_DVE_SPECS, _SUB_OPCODE_FOR_NAME, _CUSTOM_DVE_ROW_BASE,
    )
    from concourse.dve_uop import DveOpSpec

    name = "LIF_STEP_PRE_ANT"
    for op in OPS:
        if op.name == name:
            return op

    # ring stores pre-reset membrane: m = reset(prev)*DECAY + x
    u = (Src0 <= C1) * Src0
    body = u * C0 + Src1

    def ref(in0, in1, s0, s1, imm2):
        uu = (in0 * (in0 <= np.float32(s1))).astype(np.float32)
        return (uu * np.float32(s0) + in1).astype(np.float32)

    spec = Spec(body=body, reference=ref)
    opcode = _CUSTOM_DVE_ROW_BASE + len(OPS)
    shas = {}
    for ver in ("v3", "v4"):
        uops = lower(spec, ver=ver)
        shas[ver] = DveOpSpec(name=name, opcode=opcode, uops=uops, rd1_en=True).sha(ver)
    op = DveOp(name, spec, subdim=False, uops_sha=shas)
    OPS.append(op)
    _SUB_OPCODE_FOR_NAME[name] = opcode
    CUSTOM_DVE_SPECS[name] = spec
    return op


def _build():
    if "nc" in _CACHE:
        return _CACHE["nc"]
    from contextlib import ExitStack
    import concourse.bacc as bacc
    import concourse.tile as tile
    from concourse import mybir

    lif_op = _register_lif_op()

    nc = bacc.Bacc("TRN2", target_bir_lowering=False, debug=False,
                   num_devices=N_CORES)
    f32 = mybir.dt.float32
    f32r = mybir.dt.float32r
    u8 = mybir.dt.uint8
    xin_d = nc.dram_tensor("xin", [D, T * B_L], f32r, kind="ExternalInput").ap()
    wt_d = nc.dram_tensor("wt", [D, H], f32r, kind="ExternalInput").ap()
    bias_d = nc.dram_tensor("bias", [P, HT], f32, kind="ExternalInput").ap()
    out_d = nc.dram_tensor("out", [NB, P, BLK * F], u8, kind="ExternalOutput").ap()

    with tile.TileContext(nc) as tc, ExitStack() as ctx:
        const_pool = ctx.enter_context(tc.tile_pool(name="const", bufs=1))
        rhs_pool = ctx.enter_context(tc.tile_pool(name="rhs", bufs=2))
        xs_pool = ctx.enter_context(tc.tile_pool(name="xs", bufs=2))
        psum_pool = ctx.enter_context(tc.tile_pool(name="psum", bufs=1, space="PSUM"))
        spk_pool = ctx.enter_context(tc.tile_pool(name="spk", bufs=2))

        xin_r = xin_d.rearrange("(dt p) n -> p dt n", dt=DT)
        wt_r = wt_d.rearrange("(dt p) h -> dt p h", dt=DT)

        # --- W as 8 per-k-tile tiles; first one gates the first matmul ---
        wt_s = []
        for dt in range(DT):
            w_t = const_pool.tile([P, H], f32r, name=f"wt{dt}")
            nc.sync.dma_start(w_t[:], wt_r[dt])
            wt_s.append(w_t)
        bias_s = const_pool.tile([P, HT], f32)
        nc.sync.dma_start(bias_s[:], bias_d)

        # --- membrane ring: slot t%RING = pre-reset membrane after step t
        ring = const_pool.tile([P, RING * F], f32)
        nc.vector.memset(ring[:, (RING - 1) * F:], 0.0)

        # heaviside(m - TH) constants for the ScalarE sigmoid
        sig_scale = const_pool.tile([P, 1], f32)
        nc.vector.memset(sig_scale[:], SIG_SCALE)
        sig_bias = const_pool.tile([P, 1], f32)
        nc.vector.memset(sig_bias[:], -THRESH * SIG_SCALE)

        # --- PSUM: one full bank per h-tile ---
        pt = [psum_pool.tile([P, NCOL], f32, name=f"pt{ht}") for ht in range(HT)]

        for w in range(NW):
            # load input^T window: [d_lo, (dt, 64t x 8b)]  (2 MiB)
            rhs = rhs_pool.tile([P, DT * NCOL], f32r)
            if w == 0:
                # split per k-tile so dt=0 matmuls start ~2us in
                for dt in range(DT):
                    nc.sync.dma_start(
                        rhs[:, dt * NCOL:(dt + 1) * NCOL],
                        xin_r[:, dt, w * NCOL:(w + 1) * NCOL],
                    )
            else:
                nc.sync.dma_start(
                    rhs[:].rearrange("p (dt n) -> p dt n", dt=DT),
                    xin_r[:, :, w * NCOL:(w + 1) * NCOL],
                )
            # window 0: k-outer (start behind the W stream); rest: h-outer
            # (frees each bank right after its 8 k-steps -> no copy bubble)
            order = ([(dt, ht) for dt in range(DT) for ht in range(HT)] if w == 0
                     else [(dt, ht) for ht in range(HT) for dt in range(DT)])
            for dt, ht in order:
                nc.tensor.matmul(
                    pt[ht][:],
                    wt_s[dt][:, ht * P: ht * P + P],
                    rhs[:, dt * NCOL:(dt + 1) * NCOL],
                    start=(dt == 0),
                    stop=(dt == DT - 1),
                )
            # PSUM -> SBUF with bias add (ScalarE)
            xs = xs_pool.tile([P, HT * NCOL], f32)        # [p, (ht, t64, b8)]
            for ht in range(HT):
                nc.scalar.activation(
                    xs[:, ht * NCOL:(ht + 1) * NCOL],
                    pt[ht][:],
                    mybir.ActivationFunctionType.Identity,
                    bias=bias_s[:, ht:ht + 1],
                    scale=1.0,
                )
            # scan: one fused DVE op per timestep
            xs_r = xs[:].rearrange("p (ht t b) -> p t ht b", ht=HT, t=WT, b=B_L)
            for tt in range(WT):
                t = w * WT + tt
                s_out = (t % RING) * F
                s_in = ((t - 1) % RING) * F
                nc.vector._custom_dve(
                    lif_op,
                    out=ring[:, s_out:s_out + F],
                    in0=ring[:, s_in:s_in + F],
                    in1=xs_r[:, tt],
                    s0=DECAY,
                    s1=THRESH,
                )
                # spikes every BLK steps: ScalarE heaviside(m - TH) -> uint8
                if (t + 1) % BLK == 0:
                    blk = t // BLK
                    roff = ((blk * BLK) % RING) * F
                    spk = spk_pool.tile([P, BLK * F], u8)
                    nc.scalar.activation(
                        spk[:],
                        ring[:, roff:roff + BLK * F],
                        mybir.ActivationFunctionType.Sigmoid,
                        bias=sig_bias[:],
                        scale=sig_scale[:],
                    )
                    nc.sync.dma_start(out_d[blk], spk[:])

    nc.compile()
    _CACHE["nc"] = nc
    return nc


def _prep_inputs(input_data, W, b):
    """Full [B,T,D] inputs -> per-core in_maps (host-side shard + transpose)."""
    input_data = np.asarray(input_data, dtype=np.float32)
    W = np.asarray(W, dtype=np.float32)
    b = np.asarray(b, dtype=np.float32)
    wt = np.ascontiguousarray(W.T)                       # [d, h]
    bias = np.ascontiguousarray(b.reshape(HT, P).T)      # [h_lo, ht]
    in_maps = []
    for c in range(N_CORES):
        xc = input_data[c * B_L:(c + 1) * B_L]           # [8, T, D]
        xin = np.ascontiguousarray(xc.transpose(2, 1, 0)).reshape(D, T * B_L)
        in_maps.append({"xin": xin, "wt": wt, "bias": bias})
    return in_maps


def _decode_outputs(results):
    """Per-core uint8 'out' buffers -> full [B,T,H] float32."""
    outs = []
    for c in range(N_CORES):
        o = results[c]["out"]                            # [NB, P, BLK*F] u8
        o = o.reshape(NB, P, BLK, HT, B_L)               # [blk, h_lo, t, ht, b]
        o = o.transpose(4, 0, 2, 3, 1).reshape(B_L, T, H)
        outs.append(o.astype(np.float32))
    return np.ascontiguousarray(np.concatenate(outs, axis=0))


def kernel(input_data, W, b):
    from concourse.bass_utils import run_bass_kernel_spmd

    nc = _build()
    in_maps = _prep_inputs(input_data, W, b)
    res = run_bass_kernel_spmd(nc, in_maps, core_ids=list(range(N_CORES)))
    return _decode_outputs(res.results)
